# revision 26
# baseline (speedup 1.0000x reference)
"""Trainium2 Bass kernel for: Conv3d(3->16, k=3, VALID) -> min over depth -> softmax(channels).

Full inputs:  x [8, 3, 24, 128, 128] f32, conv_weight [16, 3, 3, 3, 3] f32
Full output:  [8, 16, 126, 126] f32
Sharding: data-parallel over batch, one sample per NeuronCore (8 cores).

Per-core scheme (H-packed im2col, fp16 matmuls, K padded to 128):
  - 16 h-blocks b; block handles h_out in [8b, 8b+8) (last block: 6 rows).
  - Host pre-packs x: rows r=(c*3+kw)*hh_n+hh hold x[c, d, 8b+hh, kw+w].
    Blocks 0-3 ship all 128 rows (90..127 zero) so the 4 rotating SBUF x
    tiles get their top rows zeroed by the DMA itself (garbage there could
    be Inf/NaN and 0*Inf=NaN in PSUM); blocks 4-15 ship only the 90 used
    rows (~30% less DMA than the 128-row v1 pack).
  - lhsT_p [128, M=(h_l,oc)] = W[oc, c, p, hh-h_l, kw], p=kd, host
    pre-transposed to [r,p,m] for a contiguous load. K=128 keeps FWL on.
  - Depth quads DQ (5x4 + 1x2); per quad one PSUM bank [M, nd, 126]
    accumulating the 3 kd passes. Loop order: quad-pair outer, kd inner,
    so one LDWEIGHTS feeds two N=504 matmuls (9 LDW/block instead of 18).
  - min over depth: NO tensor_reduce (1x-mode from PSUM => ~78us of DVE,
    the v1 bottleneck). ScalarE copies quads q1,q3,q4,q5 PSUM->SBUF fp16
    (Copy is in every ACT table set), DVE does a pairwise tensor_tensor
    min tree (TT reads 2 elems/cycle; fp16 SBUF legs run 2x_1P). TTs never
    read two PSUM operands (single DVE PSUM read port).
  - softmax over the 16 channels per partition group of 16, as
    exp(mn - ln(sum exp)): exp/Ln on ScalarE (both forced into the single
    natural_log_exp_and_others table set - see the patch below; without it
    bacc alternates exp_and_others/natural_log, ~9 table reloads at
    ~2.7us each on HW), channel-sum-broadcast as one fp16 PE matmul with
    the 0/1 block matrix ob, subtract on DVE fp16 2x. Pipelined across the
    conv stream in groups of (4,4,4,3,1) h-blocks.
  - Output y4[h_l, oc, block, w] fp16: ONE gpsimd-queue (SWDGE, idle Pool
    engine) DMA per softmax group, 1 descriptor/partition. scalar-queue
    DMAs would eat ~1.5us of ACT each; per-block h-major writes would be
    128 descriptors per DMA. Host de-interleaves + casts to f32.

Measured on 8xTRN2 (reps-delta method): 99.0us (v1 baseline) -> 70.7-73.0us
(run-to-run noise ~1.5us). TimelineSim attribution for the remainder: PE
busy ~63us (288 conv MMs, N=504 @ ~219ns + 5 ob MMs), startup ~4us,
softmax tail ~4us, drain ~5us.
PE work is the floor for this 3-pass-per-block mapping; a 2-pass
(oc,2h,2d,2w) im2col mapping would cut PE to ~39us but needs a 16.8MB
rhs (7.1x replication, ~47us DMA) plus ~45us of small-tile eviction -
net ~55-60us, not attempted (see session notes).
"""

import functools
import os
import sys

import numpy as np

os.environ.setdefault("MYCRO_LOCAL_CACHE", "1")
if os.path.isdir("/opt/trn_rl_repo") and "/opt/trn_rl_repo" not in sys.path:
    sys.path.insert(0, "/opt/trn_rl_repo")

import concourse.bacc as bacc
import concourse.mybir as mybir
import concourse.tile as tile
from concourse import bass_utils

# Force Exp/Ln/Copy onto the single natural_log_exp_and_others ACT table set.
# bacc's insert_act_table_loads otherwise assigns Exp -> exp_and_others and
# Ln -> natural_log, and the per-group Exp/Ln alternation then reloads the
# table ~9x per iteration (~2.7us each on HW).
import concourse.hw_specs as _hw_specs

if not getattr(_hw_specs, "_act_tables_patched", False):
    _orig_get_activation_tables = _hw_specs.get_activation_tables

    def _patched_get_activation_tables(module_arch):
        tables = {
            k: set(v) for k, v in _orig_get_activation_tables(module_arch).items()
        }
        keep = "natural_log_exp_and_others"
        exp_ln = {
            mybir.ActivationFunctionType.Exp,
            mybir.ActivationFunctionType.Ln,
        }
        if keep in tables and exp_ln <= tables[keep]:
            for name, funcs in tables.items():
                if name != keep:
                    funcs -= exp_ln
        return tables

    _patched_get_activation_tables = functools.cache(_patched_get_activation_tables)
    _hw_specs.get_activation_tables = _patched_get_activation_tables
    bacc.get_activation_tables = _patched_get_activation_tables
    _hw_specs._act_tables_patched = True

C, D, H, W = 3, 24, 128, 128
OC, KD, KH, KW = 16, 3, 3, 3
DO, HO, WO = D - 2, H - 2, W - 2  # 22, 126, 126
DQ = ((0, 4), (4, 4), (8, 4), (12, 4), (16, 4), (20, 2))  # (start, ndepth)
NCORES = 8
NBLK = 16  # h blocks: 15 full (8 rows) + 1 tail (6 rows)
NROWS = 90  # used K rows: 3c * 3kw * 10hh (tail block: 72, zero-padded to 90)
GROUPS = ((0, 4), (4, 4), (8, 4), (12, 3), (15, 1))  # softmax groups (start, size)
F32 = mybir.dt.float32
F16 = mybir.dt.float16
Act = mybir.ActivationFunctionType
Alu = mybir.AluOpType


def _pack_weights(w: np.ndarray):
    """lhsT/lhsT_last [KD,128,128] (zero-padded K and M) + ob [128,128]."""

    def pack(nh):
        hh_n = nh + 2
        lhsT = np.zeros((KD, 128, 128), dtype=np.float32)
        for p in range(KD):
            for c in range(C):
                for kw in range(KW):
                    for hh in range(hh_n):
                        r = (c * KW + kw) * hh_n + hh
                        for hl in range(nh):
                            kh = hh - hl
                            if 0 <= kh < KH:
                                lhsT[p, r, hl * OC : hl * OC + OC] = w[:, c, p, kh, kw]
        return lhsT

    ob = np.zeros((128, 128), dtype=np.float32)
    for pp in range(128):
        g0 = (pp // OC) * OC
        ob[pp, g0 : g0 + OC] = 1.0
    return pack(8), pack(6), ob


def _pack_x5(x1: np.ndarray):
    """x [3,24,128,128] f32 -> x5a [4,128,24,126] + x5b [12,90,24,126] f16."""
    x5 = np.zeros((NBLK, 128, D, WO), dtype=np.float16)
    for b in range(NBLK):
        nh = 8 if b < NBLK - 1 else 6
        hh_n = nh + 2
        for c in range(C):
            for kw in range(KW):
                r0 = (c * KW + kw) * hh_n
                # [hh, d, w] <- x[c, d, 8b+hh, kw+w]
                x5[b, r0 : r0 + hh_n] = np.transpose(
                    x1[c, :, 8 * b : 8 * b + hh_n, kw : kw + WO], (1, 0, 2)
                )
    return np.ascontiguousarray(x5[:4]), np.ascontiguousarray(x5[4:, :NROWS])


def build_program(reps: int = 1, stage2: str = "full", stage1: str = "full"):
    """reps > 1 wraps the per-sample body in a hardware loop (dev timing only).
    stage2: none | exp | smmm | full (dev bisection of the softmax tail).
    stage1: conv (PE+DMA only) | nomin (ACT copies, no DVE tree) | full."""
    nc = bacc.Bacc(
        "TRN2",
        target_bir_lowering=False,
        debug=False,
        enable_asserts=True,
        num_devices=NCORES,
    )
    # blocks 0-3 ship all 128 rows (90..127 zero) so the 4 rotating SBUF x
    # tiles never need zeroing; blocks 4-15 reuse the zeros left in rows
    # 90..127 by blocks 0-3 and ship only the 90 used rows.
    x5a_d = nc.dram_tensor("x5a", [4, 128, D, WO], F16, kind="ExternalInput").ap()
    x5_d = nc.dram_tensor("x5", [NBLK - 4, NROWS, D, WO], F16, kind="ExternalInput").ap()
    # host pre-transposed to [r, p, m] so the load is a plain contiguous DMA
    lw_d = nc.dram_tensor("lw", [128, KD, 128], F16, kind="ExternalInput").ap()
    lwl_d = nc.dram_tensor("lwl", [128, KD, 128], F16, kind="ExternalInput").ap()
    ob_d = nc.dram_tensor("ob", [128, 128], F16, kind="ExternalInput").ap()
    # y4[h_l, oc, block, w]: per softmax group, partition (h_l, oc) writes ONE
    # contiguous gsz*126 run -> 1 DMA descriptor per partition per group.
    y_d = nc.dram_tensor("y", [8, OC, NBLK, WO], F16, kind="ExternalOutput").ap()

    with tile.TileContext(nc) as tc:
        with (
            tc.tile_pool(name="const", bufs=1) as cpool,
            tc.tile_pool(name="xt", bufs=4) as xpool,
            tc.tile_pool(name="sm", bufs=3) as spool,
            tc.tile_pool(name="qps", bufs=6, space="PSUM") as qpool,
            tc.tile_pool(name="sps", bufs=2, space="PSUM") as smpool,
        ):
            # startup ordering: first conv matmul needs lw + x5[0] only.
            # lw + first half of x5[0] ride the scalar queue (ACT idle at t=0),
            # the rest of x5[0] rides sync; lwl/ob follow later on scalar.
            lw_sb = cpool.tile([128, KD, 128], F16)
            nc.scalar.dma_start(lw_sb[:], lw_d)
            lwl_sb = cpool.tile([128, KD, 128], F16)
            nc.scalar.dma_start(lwl_sb[:], lwl_d)
            ob_sb = cpool.tile([128, 128], F16)
            nc.scalar.dma_start(ob_sb[:], ob_d)



            def emit_body():
                state = {}  # per softmax group g: mn/et/st/lt/dt tiles

                def softmax_step(step, g):
                    g0, gsz = GROUPS[g]
                    if step == 0 and stage2 != "none":
                        et = spool.tile([128, gsz, WO], F16, tag="et", bufs=3, name=f"et{g}")
                        nc.scalar.activation(et[:], state[g]["mn"][:], Act.Exp)
                        state[g]["et"] = et
                    if stage2 in ("none", "exp"):
                        return
                    eg = state[g]["et"][:]
                    if step == 1:
                        # group-sum broadcast to all 128 partitions in one MM:
                        # ob[k, p] = 1 iff k//16 == p//16
                        st = smpool.tile([128, gsz, WO], F32, tag="ss", name=f"st{g}")
                        nc.tensor.matmul(st[:], ob_sb[:], eg, start=True, stop=True)
                        state[g]["st"] = st
                    elif step == 2:
                        lt = spool.tile([128, gsz, WO], F16, tag="lt", bufs=2, name=f"lt{g}")
                        nc.scalar.activation(lt[:], state[g]["st"][:], Act.Ln)
                        state[g]["lt"] = lt
                    elif step == 3:
                        dt = spool.tile([128, gsz, WO], F16, tag="dt", bufs=2, name=f"dt{g}")
                        nc.vector.tensor_tensor(
                            dt[:], state[g]["mn"][:], state[g]["lt"][:],
                            op=Alu.subtract,
                        )
                        state[g]["dt"] = dt
                    elif step == 4:
                        if stage2 == "smmm":
                            return
                        ot = spool.tile([128, gsz, WO], F16, tag="ot", bufs=2, name=f"ot{g}")
                        nc.scalar.activation(ot[:], state[g]["dt"][:], Act.Exp)
                        # one DMA per group; partition (h_l, oc) writes one
                        # contiguous gsz*126 run. SWDGE (gpsimd) queue: the
                        # otherwise-idle Pool engine issues it, keeping ACT
                        # free. Tail-block garbage partitions land in y4
                        # slots the host drops.
                        dst = y_d[:, :, g0 : g0 + gsz, :].rearrange(
                            "h oc j w -> (h oc) j w"
                        )
                        nc.gpsimd.dma_start(dst, ot[:, :, :])

                # schedule[B] = list of (step, g) to emit before conv block B
                schedule = {}
                for g, (g0, gsz) in enumerate(GROUPS):
                    end = g0 + gsz - 1
                    for step, off in enumerate((1, 4, 5, 6, 7)):
                        schedule.setdefault(end + off, []).append((step, g))

                blk2grp = {}
                for g, (g0, gsz) in enumerate(GROUPS):
                    for b in range(g0, g0 + gsz):
                        blk2grp[b] = g

                for b in range(NBLK):
                    g_cur = blk2grp[b]
                    g0, gsz = GROUPS[g_cur]
                    if b == g0:
                        state[g_cur] = {
                            "mn": spool.tile([128, gsz, WO], F16, tag="mn", bufs=3, name=f"mn{g_cur}")
                        }
                    lw_t = lw_sb if b < NBLK - 1 else lwl_sb
                    if stage1 == "full":
                        for step, g in schedule.get(b, []):
                            softmax_step(step, g)

                    xt = xpool.tile([128, D, WO], F16, tag="xt")
                    if b == 0:
                        # three piece loads on the SAME queue: the d[0:10]
                        # piece lands first so the first quad-pair matmuls
                        # start early; lw rides the scalar queue in parallel
                        nc.sync.dma_start(
                            xt[:, 0:10, :], x5a_d[0][:, 0:10, :].bitcast(F16)
                        )
                        nc.sync.dma_start(
                            xt[:, 10:18, :], x5a_d[0][:, 10:18, :].bitcast(F16)
                        )
                        nc.sync.dma_start(
                            xt[:, 18:D, :], x5a_d[0][:, 18:D, :].bitcast(F16)
                        )
                    elif b < 4:
                        nc.sync.dma_start(xt[:], x5a_d[b].bitcast(F16))
                    else:
                        nc.sync.dma_start(xt[:NROWS, :, :], x5_d[b - 4].bitcast(F16))


                    # conv matmuls: quad-pair outer, kd inner (1 LDW / 2 MMs)
                    qt = []
                    for qp in range(3):
                        pts = []
                        for q in (2 * qp, 2 * qp + 1):
                            dq, nd = DQ[q]
                            pts.append(
                                qpool.tile([128, nd, WO], F32, tag="q", name=f"pt{q}")
                            )
                        # alternate the kd order per pair so consecutive pairs
                        # share their boundary weight load (saves ~1/3 of the
                        # LDWEIGHTS); any order accumulates the same sum.
                        p_seq = (0, 1, 2) if (b * 3 + qp) % 2 == 0 else (2, 1, 0)
                        for pi, p in enumerate(p_seq):
                            for i, q in enumerate((2 * qp, 2 * qp + 1)):
                                dq, nd = DQ[q]
                                nc.tensor.matmul(
                                    pts[i][:],
                                    lw_t[:, p, :],
                                    xt[:, dq + p : dq + p + nd, :],
                                    start=(pi == 0),
                                    stop=(pi == KD - 1),
                                )
                        qt.extend(pts)

                        if stage1 == "conv":
                            continue
                        # evict as soon as each pair completes:
                        # ScalarE copies odd quads (+ q4) to fp16 SBUF; DVE
                        # never reads two PSUM operands in one TT.
                        if qp == 0:
                            c1 = spool.tile([128, 4, WO], F16, tag="c1", bufs=2)
                            nc.scalar.copy(c1[:], qt[1][:])
                            if stage1 == "full":
                                s0 = spool.tile([128, 4, WO], F16, tag="s0", bufs=2)
                                nc.vector.tensor_tensor(s0[:], qt[0][:], c1[:], op=Alu.min)
                        elif qp == 1:
                            c3 = spool.tile([128, 4, WO], F16, tag="c3", bufs=2)
                            nc.scalar.copy(c3[:], qt[3][:])
                            if stage1 == "full":
                                s1 = spool.tile([128, 4, WO], F16, tag="s1", bufs=2)
                                nc.vector.tensor_tensor(s1[:], qt[2][:], c3[:], op=Alu.min)
                                t = spool.tile([128, 4, WO], F16, tag="t", bufs=2)
                                nc.vector.tensor_tensor(t[:], s0[:], s1[:], op=Alu.min)
                                r = spool.tile([128, 2, WO], F16, tag="r", bufs=2)
                                nc.vector.tensor_tensor(
                                    r[:], t[:, 0:2, :], t[:, 2:4, :], op=Alu.min
                                )
                        else:
                            c4 = spool.tile([128, 4, WO], F16, tag="c4", bufs=2)
                            nc.scalar.copy(c4[:], qt[4][:])
                            c5 = spool.tile([128, 2, WO], F16, tag="c5", bufs=2)
                            nc.scalar.copy(c5[:], qt[5][:])
                            if stage1 == "full":
                                a = spool.tile([128, 2, WO], F16, tag="a", bufs=2)
                                nc.vector.tensor_tensor(
                                    a[:], c4[:, 0:2, :], c4[:, 2:4, :], op=Alu.min
                                )
                                b2 = spool.tile([128, 2, WO], F16, tag="b2", bufs=2)
                                nc.vector.tensor_tensor(b2[:], a[:], c5[:], op=Alu.min)
                                m2 = spool.tile([128, 2, WO], F16, tag="m2", bufs=2)
                                nc.vector.tensor_tensor(m2[:], r[:], b2[:], op=Alu.min)
                                nc.vector.tensor_tensor(
                                    state[g_cur]["mn"][:, b - g0 : b - g0 + 1, :],
                                    m2[:, 0:1, :],
                                    m2[:, 1:2, :],
                                    op=Alu.min,
                                )

                # flush softmax steps scheduled past the last conv block
                if stage1 == "full":
                    for at in sorted(k for k in schedule if k >= NBLK):
                        for step, g in schedule[at]:
                            softmax_step(step, g)
                    # HAM warmkeepers: two throwaway matmuls that execute
                    # during the softmax tail so the PE-idle window across
                    # the iteration boundary stays under the ~3.4us HAM MID
                    # window (otherwise the first ~3.4us of the next
                    # iteration's matmuls run at 1.2 GHz instead of 2.4).
                    # The second rotates onto the last st bank, so it waits
                    # for Ln(g4) and lands late in the tail by construction.
                    for wi in range(2):
                        wk = smpool.tile([128, 128], F32, tag="ss", name=f"warm{wi}")
                        nc.tensor.matmul(wk[:], ob_sb[:], ob_sb[:], start=True, stop=True)

            if reps == 1:
                emit_body()
            else:
                with tc.For_i(0, reps, 1, hint_engines=(mybir.EngineType.PE,), staggered_reset=True):
                    emit_body()

    nc.compile()
    return nc


@functools.lru_cache(maxsize=1)
def _program():
    return build_program()


def make_in_maps(x: np.ndarray, w: np.ndarray):
    lw, lwl, ob = _pack_weights(w)
    # device expects [r, p, m] (partition-major) for a contiguous DMA
    lw = np.ascontiguousarray(lw.transpose(1, 0, 2)).astype(np.float16)
    lwl = np.ascontiguousarray(lwl.transpose(1, 0, 2)).astype(np.float16)
    maps = []
    for i in range(x.shape[0]):
        x5a, x5b = _pack_x5(x[i])
        maps.append(
            {"x5a": x5a, "x5": x5b, "lw": lw, "lwl": lwl, "ob": ob.astype(np.float16)}
        )
    return maps


def kernel(x, conv_weight):
    x = np.ascontiguousarray(np.asarray(x, dtype=np.float32))
    w = np.ascontiguousarray(np.asarray(conv_weight, dtype=np.float32))
    assert x.shape == (NCORES, C, D, H, W), x.shape
    nc = _program()
    in_maps = make_in_maps(x, w)
    res = bass_utils.run_bass_kernel_spmd(nc, in_maps, core_ids=list(range(NCORES)))
    # y4 [h_l, oc, block, w] fp16 -> [oc, 8*block+h_l, w] f32 (drop the tail
    # block's h_l >= 6 garbage rows via [:HO])
    out = np.stack(
        [
            res.results[i]["y"]
            .transpose(1, 2, 0, 3)
            .reshape(OC, 8 * NBLK, WO)[:, :HO, :]
            for i in range(NCORES)
        ]
    )
    return np.ascontiguousarray(out.astype(np.float32))
